# revision 8
# baseline (speedup 1.0000x reference)
"""Trainium2 Bass kernel for soft K-means assignment (vq_codebook).

reference computes, per sample row x_n (D=256) against K=512 centroids:
    dists[n,k] = ||x_n||^2 - 2 x_n.c_k + ||c_k||^2
    out[n,k]   = softmax_k(-dists[n,k] / T),  T = 0.1

softmax is invariant to per-row constants, so ||x||^2 drops out:
    out[n,:] = softmax_k((2 x.c_k - ||c_k||^2) / T)

Strategy (8 cores, data-parallel over the flattened sample axis):
  - each core handles N_PER_CORE = 4096 rows; centroids replicated
  - centroids transposed once on-chip (PE transpose) to cT [d, k] layout
  - per 128-row tile: PE-transpose x tile, 2 accumulating fp32 matmuls
    (contraction d = 2 x 128) -> cross in PSUM, then fused DVE
    (c_sq/2 - cross) * (2/T) with min-reduce, ACT exp(+accum sum),
    reciprocal, scale, DMA out.
"""

import numpy as np
from contextlib import ExitStack

import concourse.bass as bass
import concourse.bacc as bacc
import concourse.mybir as mybir
import concourse.tile as tile
from concourse.bass_utils import run_bass_kernel_spmd
from concourse.masks import make_identity

N_CORES = 8
B, S, D = 32, 1024, 256
K = 512
N_TOTAL = B * S              # 32768
N_PER_CORE = N_TOTAL // N_CORES  # 4096
P = 128                      # partitions / rows per tile
N_TILES = N_PER_CORE // P    # 32
TEMPERATURE = 0.1

F32 = mybir.dt.float32
# Matmul compute dtype: float32 (exact) or float32r (fast, reduced precision)
MM_DT = F32


def _mm(ap, dt):
    return ap.bitcast(dt) if dt != F32 else ap


def build_program(mm_dt=MM_DT):
    nc = bacc.Bacc("TRN2", target_bir_lowering=False, debug=False)
    x_in = nc.dram_tensor("x", [N_PER_CORE, D], F32, kind="ExternalInput")
    c_in = nc.dram_tensor("centroids", [K, D], F32, kind="ExternalInput")
    out = nc.dram_tensor("out", [N_PER_CORE, K], F32, kind="ExternalOutput")

    n_kchunks = K // P   # 4
    n_dchunks = D // P   # 2

    with tile.TileContext(nc) as tc, ExitStack() as ctx:
        singles = ctx.enter_context(tc.tile_pool(name="singles", bufs=1))

        identity = singles.tile([P, P], F32)
        make_identity(nc, identity[:])

        # cT[j] holds centroids.T slice [d = 128j..128j+127, k = 0..511]
        cT = [singles.tile([P, K], F32, tag=f"cT{j}", name=f"cT{j}")
              for j in range(n_dchunks)]
        bias_bcast = singles.tile([P, K], F32)   # c_sq/2 replicated on rows
        ones_col = singles.tile([P, 1], F32)
        nc.vector.memset(ones_col[:], 1.0)

        # ---- setup: transpose centroids, compute c_sq/2 row, broadcast ----
        with tc.tile_pool(name="setup_sb", bufs=1) as setup_sb, \
             tc.tile_pool(name="setup_ps", bufs=2, space="PSUM") as setup_ps:
            c_all = setup_sb.tile([P, n_kchunks, D], F32)
            nc.sync.dma_start(
                out=c_all[:],
                in_=c_in.ap().rearrange("(c p) d -> p c d", c=n_kchunks),
            )
            for cchunk in range(n_kchunks):
                for j in range(n_dchunks):
                    ptr = setup_ps.tile([P, P], F32, tag="ptr")
                    nc.tensor.transpose(
                        ptr[:], c_all[:, cchunk, j * P:(j + 1) * P], identity[:]
                    )
                    nc.vector.tensor_copy(
                        cT[j][:, cchunk * P:(cchunk + 1) * P], ptr[:]
                    )

            sq = [setup_sb.tile([P, K], F32, tag=f"sq{j}", name=f"sq{j}")
                  for j in range(n_dchunks)]
            for j in range(n_dchunks):
                nc.scalar.square(sq[j][:], cT[j][:])
            csq_ps = setup_ps.tile([1, K], F32, tag="csq")
            for j in range(n_dchunks):
                nc.tensor.matmul(csq_ps[:], ones_col[:], sq[j][:],
                                 start=(j == 0), stop=(j == n_dchunks - 1))
            # bias_row = csq / 2   (nl = csq/2 - cross; logits = -20*nl)
            bias_row = setup_sb.tile([1, K], F32)
            nc.scalar.mul(bias_row[:], csq_ps[:], 0.5)
            # broadcast to all partitions via DRAM round-trip (step-0 DMA)
            with tc.tile_pool(name="setup_dram", bufs=1, space="DRAM") as sdram:
                bias_dram = sdram.tile([1, K], F32)
                nc.gpsimd.dma_start(out=bias_dram[:], in_=bias_row[:])
                nc.gpsimd.dma_start(out=bias_bcast[:],
                                    in_=bias_dram[:].to_broadcast([P, K]))

        # ---- main loop over 128-row tiles ----
        work = ctx.enter_context(tc.tile_pool(name="work", bufs=3))
        psum = ctx.enter_context(tc.tile_pool(name="psum", bufs=2, space="PSUM"))
        stats = ctx.enter_context(tc.tile_pool(name="stats", bufs=4))

        for t in range(N_TILES):
            rows = slice(t * P, (t + 1) * P)
            x_sb = work.tile([P, D], F32, tag="x")
            nc.sync.dma_start(out=x_sb[:], in_=x_in.ap()[rows, :])

            xT = []
            for j in range(n_dchunks):
                pt = psum.tile([P, P], F32, tag=f"pt{j}")
                nc.tensor.transpose(pt[:], x_sb[:, j * P:(j + 1) * P],
                                    identity[:])
                xt = work.tile([P, P], F32, tag=f"xT{j}")
                nc.vector.tensor_copy(xt[:], pt[:])
                xT.append(xt)

            u_ps = psum.tile([P, K], F32, tag="u")
            for j in range(n_dchunks):
                nc.tensor.matmul(u_ps[:], _mm(xT[j][:], mm_dt),
                                 _mm(cT[j][:], mm_dt),
                                 start=(j == 0), stop=(j == n_dchunks - 1))

            # nl = csq/2 - cross ; mn = min_k nl  (logits = -20*nl)
            nl = work.tile([P, K], F32, tag="nl")
            nc.vector.tensor_tensor(out=nl[:], in0=bias_bcast[:], in1=u_ps[:],
                                    op=mybir.AluOpType.subtract)
            mn = stats.tile([P, 1], F32, tag="mn")
            nc.vector.tensor_reduce(out=mn[:], in_=nl[:],
                                    axis=mybir.AxisListType.X,
                                    op=mybir.AluOpType.min)
            mn20 = stats.tile([P, 1], F32, tag="mn20")
            nc.vector.tensor_scalar_mul(mn20[:], mn[:], 2.0 / TEMPERATURE)

            # e = exp(-20*nl + 20*mn); s = sum_k e  (ACT pass with accumulate)
            e_sb = work.tile([P, K], F32, tag="e")
            s_sb = stats.tile([P, 1], F32, tag="s")
            nc.scalar.activation(e_sb[:], nl[:],
                                 mybir.ActivationFunctionType.Exp,
                                 bias=mn20[:], scale=-2.0 / TEMPERATURE,
                                 accum_out=s_sb[:])

            r_sb = stats.tile([P, 1], F32, tag="r")
            nc.vector.reciprocal(r_sb[:], s_sb[:])

            o_sb = work.tile([P, K], F32, tag="o")
            nc.vector.tensor_scalar_mul(o_sb[:], e_sb[:], r_sb[:])
            nc.sync.dma_start(out=out.ap()[rows, :], in_=o_sb[:])

    nc.compile()
    return nc


_CACHED_NC = None


def kernel(x, centroids):
    global _CACHED_NC
    if _CACHED_NC is None:
        _CACHED_NC = build_program()
    nc = _CACHED_NC

    xf = np.ascontiguousarray(np.asarray(x, dtype=np.float32)
                              .reshape(N_TOTAL, D))
    cf = np.ascontiguousarray(np.asarray(centroids, dtype=np.float32))
    in_maps = [
        {"x": xf[i * N_PER_CORE:(i + 1) * N_PER_CORE], "centroids": cf}
        for i in range(N_CORES)
    ]
    res = run_bass_kernel_spmd(nc, in_maps, core_ids=list(range(N_CORES)))
    outs = np.concatenate([r["out"] for r in res.results], axis=0)
    return outs.reshape(B, S, K)


# revision 10
# speedup vs baseline: 28974.3365x; 28974.3365x over previous
"""Trainium2 Bass kernel for soft K-means assignment (vq_codebook).

reference computes, per sample row x_n (D=256) against K=512 centroids:
    dists[n,k] = ||x_n||^2 - 2 x_n.c_k + ||c_k||^2
    out[n,k]   = softmax_k(-dists[n,k] / T),  T = 0.1

softmax is invariant to per-row constants, so ||x||^2 drops out:
    out[n,:] = softmax_k((2 x.c_k - ||c_k||^2) / T)

Strategy (8 cores, data-parallel over the flattened sample axis):
  - each core handles N_PER_CORE = 4096 rows; centroids replicated
  - centroids transposed once on-chip (PE transpose) to cT [d, k] layout
  - per 128-row tile: PE-transpose x tile, 2 accumulating fp32 matmuls
    (contraction d = 2 x 128) -> cross in PSUM, then fused DVE
    (c_sq/2 - cross) * (2/T) with min-reduce, ACT exp(+accum sum),
    reciprocal, scale, DMA out.
"""

import numpy as np
from contextlib import ExitStack

import concourse.bass as bass
import concourse.bacc as bacc
import concourse.mybir as mybir
import concourse.tile as tile
from concourse.bass_utils import run_bass_kernel_spmd
from concourse.masks import make_identity

N_CORES = 8
B, S, D = 32, 1024, 256
K = 512
N_TOTAL = B * S              # 32768
N_PER_CORE = N_TOTAL // N_CORES  # 4096
P = 128                      # partitions / rows per tile
N_TILES = N_PER_CORE // P    # 32
TEMPERATURE = 0.1

F32 = mybir.dt.float32
# Matmul compute dtype: float32 (exact) or float32r (fast, reduced precision)
MM_DT = F32


def _mm(ap, dt):
    return ap.bitcast(dt) if dt != F32 else ap


def build_program(mm_dt=MM_DT):
    nc = bacc.Bacc("TRN2", target_bir_lowering=False, debug=False)
    x_in = nc.dram_tensor("x", [N_PER_CORE, D], F32, kind="ExternalInput")
    c_in = nc.dram_tensor("centroids", [K, D], F32, kind="ExternalInput")
    out = nc.dram_tensor("out", [N_PER_CORE, K], F32, kind="ExternalOutput")

    n_kchunks = K // P   # 4
    n_dchunks = D // P   # 2

    with tile.TileContext(nc) as tc, ExitStack() as ctx:
        singles = ctx.enter_context(tc.tile_pool(name="singles", bufs=1))

        identity = singles.tile([P, P], F32)
        make_identity(nc, identity[:])

        # cT[j] holds centroids.T slice [d = 128j..128j+127, k = 0..511]
        cT = [singles.tile([P, K], F32, tag=f"cT{j}", name=f"cT{j}")
              for j in range(n_dchunks)]
        bias_bcast = singles.tile([P, K], F32)   # c_sq/2 replicated on rows
        ones_col = singles.tile([P, 1], F32)
        nc.vector.memset(ones_col[:], 1.0)

        # ---- setup: transpose centroids, compute c_sq/2 row, broadcast ----
        with tc.tile_pool(name="setup_sb", bufs=1) as setup_sb, \
             tc.tile_pool(name="setup_ps", bufs=2, space="PSUM") as setup_ps:
            c_all = setup_sb.tile([P, n_kchunks, D], F32)
            nc.sync.dma_start(
                out=c_all[:],
                in_=c_in.ap().rearrange("(c p) d -> p c d", c=n_kchunks),
            )
            for cchunk in range(n_kchunks):
                for j in range(n_dchunks):
                    ptr = setup_ps.tile([P, P], F32, tag="ptr")
                    nc.tensor.transpose(
                        ptr[:], c_all[:, cchunk, j * P:(j + 1) * P], identity[:]
                    )
                    nc.vector.tensor_copy(
                        cT[j][:, cchunk * P:(cchunk + 1) * P], ptr[:]
                    )

            sq = [setup_sb.tile([P, K], F32, tag=f"sq{j}", name=f"sq{j}")
                  for j in range(n_dchunks)]
            for j in range(n_dchunks):
                nc.scalar.square(sq[j][:], cT[j][:])
            csq_ps = setup_ps.tile([1, K], F32, tag="csq")
            for j in range(n_dchunks):
                nc.tensor.matmul(csq_ps[:], ones_col[:], sq[j][:],
                                 start=(j == 0), stop=(j == n_dchunks - 1))
            # bias_row = csq / 2   (nl = csq/2 - cross; logits = -20*nl)
            bias_row = setup_sb.tile([1, K], F32)
            nc.scalar.mul(bias_row[:], csq_ps[:], 0.5)
            # broadcast to all partitions via DRAM round-trip (step-0 DMA)
            with tc.tile_pool(name="setup_dram", bufs=1, space="DRAM") as sdram:
                bias_dram = sdram.tile([1, K], F32)
                nc.gpsimd.dma_start(out=bias_dram[:], in_=bias_row[:])
                nc.gpsimd.dma_start(out=bias_bcast[:],
                                    in_=bias_dram[:].to_broadcast([P, K]))

        # ---- main loop over 128-row tiles ----
        work = ctx.enter_context(tc.tile_pool(name="work", bufs=5))
        psum = ctx.enter_context(tc.tile_pool(name="psum", bufs=2, space="PSUM"))
        stats = ctx.enter_context(tc.tile_pool(name="stats", bufs=8))

        for t in range(N_TILES):
            rows = slice(t * P, (t + 1) * P)
            x_sb = work.tile([P, D], F32, tag="x")
            nc.sync.dma_start(out=x_sb[:], in_=x_in.ap()[rows, :])

            xT = []
            for j in range(n_dchunks):
                pt = psum.tile([P, P], F32, tag=f"pt{j}")
                nc.tensor.transpose(pt[:], x_sb[:, j * P:(j + 1) * P],
                                    identity[:])
                xt = work.tile([P, P], F32, tag=f"xT{j}")
                nc.vector.tensor_copy(xt[:], pt[:])
                xT.append(xt)

            u_ps = psum.tile([P, K], F32, tag="u", bufs=3)
            for j in range(n_dchunks):
                nc.tensor.matmul(u_ps[:], _mm(xT[j][:], mm_dt),
                                 _mm(cT[j][:], mm_dt),
                                 start=(j == 0), stop=(j == n_dchunks - 1))

            # nl = csq/2 - cross ; mn = min_k nl  (logits = -20*nl)
            nl = work.tile([P, K], F32, tag="nl")
            nc.vector.tensor_tensor(out=nl[:], in0=bias_bcast[:], in1=u_ps[:],
                                    op=mybir.AluOpType.subtract)
            mn = stats.tile([P, 1], F32, tag="mn")
            nc.vector.tensor_reduce(out=mn[:], in_=nl[:],
                                    axis=mybir.AxisListType.X,
                                    op=mybir.AluOpType.min)
            mn20 = stats.tile([P, 1], F32, tag="mn20")
            nc.vector.tensor_scalar_mul(mn20[:], mn[:], 2.0 / TEMPERATURE)

            # e = exp(-20*nl + 20*mn); s = sum_k e  (ACT pass with accumulate)
            e_sb = work.tile([P, K], F32, tag="e")
            s_sb = stats.tile([P, 1], F32, tag="s")
            nc.scalar.activation(e_sb[:], nl[:],
                                 mybir.ActivationFunctionType.Exp,
                                 bias=mn20[:], scale=-2.0 / TEMPERATURE,
                                 accum_out=s_sb[:])

            r_sb = stats.tile([P, 1], F32, tag="r")
            nc.vector.reciprocal(r_sb[:], s_sb[:])

            o_sb = work.tile([P, K], F32, tag="o")
            nc.vector.tensor_scalar_mul(o_sb[:], e_sb[:], r_sb[:])
            nc.sync.dma_start(out=out.ap()[rows, :], in_=o_sb[:])

    nc.compile()
    return nc


_CACHED_NC = None


def kernel(x, centroids):
    global _CACHED_NC
    if _CACHED_NC is None:
        _CACHED_NC = build_program()
    nc = _CACHED_NC

    xf = np.ascontiguousarray(np.asarray(x, dtype=np.float32)
                              .reshape(N_TOTAL, D))
    cf = np.ascontiguousarray(np.asarray(centroids, dtype=np.float32))
    in_maps = [
        {"x": xf[i * N_PER_CORE:(i + 1) * N_PER_CORE], "centroids": cf}
        for i in range(N_CORES)
    ]
    res = run_bass_kernel_spmd(nc, in_maps, core_ids=list(range(N_CORES)))
    outs = np.concatenate([r["out"] for r in res.results], axis=0)
    return outs.reshape(B, S, K)


# revision 12
# speedup vs baseline: 31159.4088x; 1.0754x over previous
"""Trainium2 Bass kernel for soft K-means assignment (vq_codebook).

reference computes, per sample row x_n (D=256) against K=512 centroids:
    dists[n,k] = ||x_n||^2 - 2 x_n.c_k + ||c_k||^2
    out[n,k]   = softmax_k(-dists[n,k] / T),  T = 0.1

softmax is invariant to per-row constants, so ||x||^2 drops out:
    out[n,:] = softmax_k((2 x.c_k - ||c_k||^2) / T)

Strategy (8 cores, data-parallel over the flattened sample axis):
  - each core handles N_PER_CORE = 4096 rows; centroids replicated
  - centroids transposed once on-chip (PE transpose) to cT [d, k] layout
  - per 128-row tile: PE-transpose x tile (identity matmul), 2
    accumulating fp32 matmuls (contraction d = 2 x 128) -> cross in PSUM;
    DVE: nl = c_sq/2 - cross, mn = min_k nl; ACT: e = exp(-20*nl + 20*mn)
    with accumulated row sum; DVE: reciprocal + scale; DMA out.
  - note: tensor_tensor_reduce / scalar_tensor_tensor / negated reduce /
    ACT copy-with-scale-AP all misbehave or crash through this runtime's
    codegen path (verified empirically); only the op set used here is
    hardware-proven at full 32-tile scale.
"""

import numpy as np
from contextlib import ExitStack

import concourse.bass as bass
import concourse.bacc as bacc
import concourse.mybir as mybir
import concourse.tile as tile
from concourse.bass_utils import run_bass_kernel_spmd
from concourse.masks import make_identity

N_CORES = 8
B, S, D = 32, 1024, 256
K = 512
N_TOTAL = B * S              # 32768
N_PER_CORE = N_TOTAL // N_CORES  # 4096
P = 128                      # partitions / rows per tile
N_TILES = N_PER_CORE // P    # 32
TEMPERATURE = 0.1

F32 = mybir.dt.float32
# Matmul compute dtype: float32 (exact) or float32r (fast, reduced precision)
MM_DT = F32


def _mm(ap, dt):
    return ap.bitcast(dt) if dt != F32 else ap


def build_program(mm_dt=MM_DT):
    nc = bacc.Bacc("TRN2", target_bir_lowering=False, debug=False)
    # x arrives HOST-PRE-TRANSPOSED: [D, N_PER_CORE] so d lands on
    # partitions with no on-chip transpose (PE matmul contracts partitions)
    x_in = nc.dram_tensor("x", [D, N_PER_CORE], F32, kind="ExternalInput")
    c_in = nc.dram_tensor("centroids", [K, D], F32, kind="ExternalInput")
    out = nc.dram_tensor("out", [N_PER_CORE, K], F32, kind="ExternalOutput")

    n_kchunks = K // P   # 4
    n_dchunks = D // P   # 2

    with tile.TileContext(nc) as tc, ExitStack() as ctx:
        singles = ctx.enter_context(tc.tile_pool(name="singles", bufs=1))

        identity = singles.tile([P, P], F32)
        make_identity(nc, identity[:])

        # cT[j] holds centroids.T slice [d = 128j..128j+127, k = 0..511]
        cT = [singles.tile([P, K], F32, tag=f"cT{j}", name=f"cT{j}")
              for j in range(n_dchunks)]
        bias_bcast = singles.tile([P, K], F32)   # c_sq/2 replicated on rows
        ones_col = singles.tile([P, 1], F32)
        nc.vector.memset(ones_col[:], 1.0)

        # ---- setup: transpose centroids, compute c_sq/2 row, broadcast ----
        with tc.tile_pool(name="setup_sb", bufs=1) as setup_sb, \
             tc.tile_pool(name="setup_ps", bufs=2, space="PSUM") as setup_ps:
            c_all = setup_sb.tile([P, n_kchunks, D], F32)
            nc.sync.dma_start(
                out=c_all[:],
                in_=c_in.ap().rearrange("(c p) d -> p c d", c=n_kchunks),
            )
            for cchunk in range(n_kchunks):
                for j in range(n_dchunks):
                    ptr = setup_ps.tile([P, P], F32, tag="ptr")
                    nc.tensor.transpose(
                        ptr[:], c_all[:, cchunk, j * P:(j + 1) * P], identity[:]
                    )
                    nc.vector.tensor_copy(
                        cT[j][:, cchunk * P:(cchunk + 1) * P], ptr[:]
                    )

            sq = [setup_sb.tile([P, K], F32, tag=f"sq{j}", name=f"sq{j}")
                  for j in range(n_dchunks)]
            for j in range(n_dchunks):
                nc.scalar.square(sq[j][:], cT[j][:])
            csq_ps = setup_ps.tile([1, K], F32, tag="csq")
            for j in range(n_dchunks):
                nc.tensor.matmul(csq_ps[:], ones_col[:], sq[j][:],
                                 start=(j == 0), stop=(j == n_dchunks - 1))
            # bias_row = csq / 2   (nl = csq/2 - cross; logits = -20*nl)
            bias_row = setup_sb.tile([1, K], F32)
            nc.scalar.mul(bias_row[:], csq_ps[:], 0.5)
            # broadcast to all partitions via DRAM round-trip (step-0 DMA)
            with tc.tile_pool(name="setup_dram", bufs=1, space="DRAM") as sdram:
                bias_dram = sdram.tile([1, K], F32)
                nc.gpsimd.dma_start(out=bias_dram[:], in_=bias_row[:])
                nc.gpsimd.dma_start(out=bias_bcast[:],
                                    in_=bias_dram[:].to_broadcast([P, K]))

        # ---- main loop over 128-row tiles ----
        work = ctx.enter_context(tc.tile_pool(name="work", bufs=5))
        psum = ctx.enter_context(tc.tile_pool(name="psum", bufs=2, space="PSUM"))
        stats = ctx.enter_context(tc.tile_pool(name="stats", bufs=8))

        for t in range(N_TILES):
            rows = slice(t * P, (t + 1) * P)
            # load both d-chunks of the pre-transposed tile in one DMA:
            # x_sb[p, j, n] = xT[j*128 + p, t*128 + n]
            x_sb = work.tile([P, n_dchunks, P], F32, tag="x")
            nc.sync.dma_start(
                out=x_sb[:],
                in_=x_in.ap()[:, rows].rearrange("(j p) n -> p j n",
                                                 j=n_dchunks))

            u_ps = psum.tile([P, K], F32, tag="u", bufs=4)
            for j in range(n_dchunks):
                nc.tensor.matmul(u_ps[:], _mm(x_sb[:, j, :], mm_dt),
                                 _mm(cT[j][:], mm_dt),
                                 start=(j == 0), stop=(j == n_dchunks - 1))

            # nl = csq/2 - cross ; mn = min_k nl  (logits = -20*nl)
            nl = work.tile([P, K], F32, tag="nl")
            nc.vector.tensor_tensor(out=nl[:], in0=bias_bcast[:], in1=u_ps[:],
                                    op=mybir.AluOpType.subtract)
            mn = stats.tile([P, 1], F32, tag="mn")
            nc.vector.tensor_reduce(out=mn[:], in_=nl[:],
                                    axis=mybir.AxisListType.X,
                                    op=mybir.AluOpType.min)
            mn20 = stats.tile([P, 1], F32, tag="mn20")
            nc.vector.tensor_scalar_mul(mn20[:], mn[:], 2.0 / TEMPERATURE)

            # e = exp(-20*nl + 20*mn); s = sum_k e  (ACT pass with accumulate)
            e_sb = work.tile([P, K], F32, tag="e")
            s_sb = stats.tile([P, 1], F32, tag="s")
            nc.scalar.activation(e_sb[:], nl[:],
                                 mybir.ActivationFunctionType.Exp,
                                 bias=mn20[:], scale=-2.0 / TEMPERATURE,
                                 accum_out=s_sb[:])

            r_sb = stats.tile([P, 1], F32, tag="r")
            nc.vector.reciprocal(r_sb[:], s_sb[:])

            o_sb = work.tile([P, K], F32, tag="o")
            nc.vector.tensor_scalar_mul(o_sb[:], e_sb[:], r_sb[:])
            nc.sync.dma_start(out=out.ap()[rows, :], in_=o_sb[:])

    nc.compile()
    return nc


_CACHED_NC = None


def kernel(x, centroids):
    global _CACHED_NC
    if _CACHED_NC is None:
        _CACHED_NC = build_program()
    nc = _CACHED_NC

    xf = np.asarray(x, dtype=np.float32).reshape(N_TOTAL, D)
    cf = np.ascontiguousarray(np.asarray(centroids, dtype=np.float32))
    in_maps = [
        {"x": np.ascontiguousarray(
            xf[i * N_PER_CORE:(i + 1) * N_PER_CORE].T),
         "centroids": cf}
        for i in range(N_CORES)
    ]
    res = run_bass_kernel_spmd(nc, in_maps, core_ids=list(range(N_CORES)))
    outs = np.concatenate([r["out"] for r in res.results], axis=0)
    return outs.reshape(B, S, K)
